# revision 13
# baseline (speedup 1.0000x reference)
"""Trainium2 Bass kernel for CardAwarePolicy (counts-reformulated MHA + folded MLPs).

Self-contained: takes full unsharded inputs, shards batch across 8 NeuronCores
(pure data parallel), runs a Tile/Bass kernel per core, gathers the output.

Math summary (per batch element, validated against the reference in numpy):
  The masked 4-head self-attention over the 8 hand slots depends on the hand
  only through its card-count vector n[c] (c in 0..53):
      den  = EG0 @ Nsc            (per head, Nsc = n/len, 54 query-cards)
      T    = Nsc / den            (per head)
      w2   = EG0^T @ T
      Y    = w2 * Nsc
      hand-term of ctx1 = BIG @ Y (BIG folds the V-table and out_w/ctx_w1)
  The game-state/discard encoders (tiny MLPs) and the enemy embedding gather
  run on the host; their 25-row contribution to ctx1 rides under Y's two
  halves (rows 108:128 of the A half, 108:113 of the B half) so no separate
  matmul is needed.  The action scorer (5 groups of 4 actions x 32 hidden)
  runs on-device; the final +sc_b2 / invalid-action mask is applied on host.

v2: all-bf16 weights+activations (PE at 1 col/cycle without per-matmul fp32
weight reloads), 4-tile groups of 256 columns (LDWEIGHTS amortized 4x, PSUM
fits exactly in 8 banks with ctx1/u4 bank reuse), elementwise ops span the
full 1024-column group, DMAs issued from the idle sync sequencer.
"""

import sys
import numpy as np
import ml_dtypes

sys.path.insert(0, "/opt/trn_rl_repo")

BF16 = ml_dtypes.bfloat16
B_FULL = 65536
N_CORES = 8
BC = B_FULL // N_CORES        # 8192 per core
TN = 256                      # matmul free dim (PSUM quarter-bank pairs)
GN = 1024                     # columns per group (4 matmul subtiles)
NG = BC // GN                 # 8 groups per core
NH, HD, E, HS, A = 4, 3, 12, 8, 20

_CACHE = {}


# ---------------------------------------------------------------- host folding
def _fold_tables(inp):
    f = lambda k: np.asarray(inp[k], np.float64)
    card_emb = f("card_emb")
    in_w, in_b = f("in_w"), f("in_b")
    out_w, out_b = f("out_w"), f("out_b")
    ctx_w1, ctx_b1 = f("ctx_w1"), f("ctx_b1")
    ctx_w2, ctx_b2 = f("ctx_w2"), f("ctx_b2")
    sc_w1, sc_b1, sc_w2 = f("sc_w1"), f("sc_b1"), f("sc_w2")
    aci = np.asarray(inp["action_card_indices"])

    Tq = card_emb @ in_w[0:12].T + in_b[0:12]
    Tk = card_emb @ in_w[12:24].T + in_b[12:24]
    Tv = card_emb @ in_w[24:36].T + in_b[24:36]
    G = np.zeros((NH, 54, 54))
    for h in range(NH):
        G[h] = (Tq[:, 3 * h:3 * h + 3] @ Tk[:, 3 * h:3 * h + 3].T) / np.sqrt(HD)
    EG0 = np.exp(G - G.max(axis=2, keepdims=True))
    EG0[:, :, 0] = 0.0

    T = {}

    def den_lhsT(heads):
        out = np.zeros((54, 108))
        for j, h in enumerate(heads):
            out[:, 54 * j:54 * j + 54] = EG0[h].T
        return out

    def w2_lhsT(heads):
        out = np.zeros((108, 108))
        for j, h in enumerate(heads):
            out[54 * j:54 * j + 54, 54 * j:54 * j + 54] = EG0[h]
        return out

    T["t_denA"], T["t_denB"] = den_lhsT((0, 1)), den_lhsT((2, 3))
    T["t_w2A"], T["t_w2B"] = w2_lhsT((0, 1)), w2_lhsT((2, 3))

    W1hh = ctx_w1[:, 0:12] @ out_w
    u0 = 8.0 * (ctx_w1[:, 0:12] @ out_b)

    def big_lhsT(heads):
        out = np.zeros((108, 128))
        for j, h in enumerate(heads):
            out[54 * j:54 * j + 54, :] = Tv[:, 3 * h:3 * h + 3] @ W1hh[:, 3 * h:3 * h + 3].T
        return out

    # extra 25 rows: enemy-embed (12), host-computed g,d (12), rlen->u0 (1);
    # rows 0:20 ride under Y's A half, rows 20:25 under the B half.
    t_extra = np.zeros((25, 128))
    t_extra[0:12] = ctx_w1[:, 12:24].T
    t_extra[12:18] = ctx_w1[:, 24:30].T
    t_extra[18:24] = ctx_w1[:, 30:36].T
    t_extra[24] = u0
    bigA = np.zeros((128, 128))
    bigA[0:108] = big_lhsT((0, 1))
    bigA[108:128] = t_extra[0:20]
    bigB = np.zeros((113, 128))
    bigB[0:108] = big_lhsT((2, 3))
    bigB[108:113] = t_extra[20:25]
    T["t_bigA"], T["t_bigB"] = bigA, bigB

    T["b_ctx1"] = ctx_b1[:, None]

    W_uc = sc_w1[:, 0:128] @ ctx_w2
    t_uc4 = np.zeros((128, 128))
    for a in range(4):
        t_uc4[:, 32 * a:32 * a + 32] = W_uc.T
    T["t_uc4"] = t_uc4

    am = (aci != 0).astype(np.float64)
    cnt = np.maximum(am.sum(axis=1), 1.0)
    arep = (card_emb[aci] * am[:, :, None]).sum(axis=1) / cnt[:, None]
    v = arep @ sc_w1[:, 128:140].T + sc_b1 + sc_w1[:, 0:128] @ ctx_b2  # [20,32]
    b_H = np.zeros((128, 5))
    for g in range(5):
        for a in range(4):
            b_H[32 * a:32 * a + 32, g] = v[4 * g + a]
    T["b_H"] = b_H

    for g in range(5):
        t = np.zeros((128, 20))
        for a in range(4):
            t[32 * a:32 * a + 32, 4 * g + a] = sc_w2[0]
        T[f"t_sc{g}"] = t
    return T


# weight blob (bf16): each lhsT table at a column offset, base partition 0
BLOB_LAYOUT = [  # name, rows, cols
    ("t_denA", 54, 108), ("t_denB", 54, 108),
    ("t_w2A", 108, 108), ("t_w2B", 108, 108),
    ("t_bigA", 128, 128), ("t_bigB", 113, 128),
    ("t_uc4", 128, 128),
    ("t_sc0", 128, 20), ("t_sc1", 128, 20), ("t_sc2", 128, 20),
    ("t_sc3", 128, 20), ("t_sc4", 128, 20),
]
BLOB_COLS = sum(c for _, _, c in BLOB_LAYOUT)
BIAS_LAYOUT = [("b_ctx1", 128, 1), ("b_H", 128, 5)]
BIAS_COLS = sum(c for _, _, c in BIAS_LAYOUT)


def _pack_blobs(T):
    wb = np.zeros((128, BLOB_COLS), BF16)
    off = 0
    for name, rows, cols in BLOB_LAYOUT:
        wb[0:rows, off:off + cols] = T[name].astype(BF16)
        off += cols
    bb = np.zeros((128, BIAS_COLS), np.float32)
    off = 0
    for name, rows, cols in BIAS_LAYOUT:
        bb[0:rows, off:off + cols] = T[name]
        off += cols
    return wb, bb


# ---------------------------------------------------------------- bass module
def _build_module(bc):
    import concourse.bass as bass
    import concourse.bacc as bacc
    import concourse.mybir as mybir
    from concourse import tile

    dt = mybir.dt
    f32, bf16 = dt.float32, dt.bfloat16
    Alu = mybir.AluOpType
    Act = mybir.ActivationFunctionType
    ng = bc // GN
    nsg = ng // 2                 # super-groups of 2 for the score output

    nc = bacc.Bacc("TRN2", target_bir_lowering=False, debug=False)

    din = lambda name, shape, dtype: nc.dram_tensor(name, list(shape), dtype, kind="ExternalInput").ap()
    wb_d = din("wblob", (128, BLOB_COLS), bf16)
    bb_d = din("bblob", (128, BIAS_COLS), f32)
    nsc_d = din("nsc", (ng, 108, 2 * GN), bf16)   # nsc duplicated along free
    exA_d = din("exA", (ng, 20, GN), bf16)
    exB_d = din("exB", (ng, 5, GN), bf16)
    out_d = nc.dram_tensor("out", [nsg, 52, GN], f32, kind="ExternalOutput").ap()

    with tile.TileContext(nc) as tc:
        with (
            tc.tile_pool(name="const", bufs=1) as cpool,
            tc.tile_pool(name="io", bufs=2) as io,
            tc.tile_pool(name="work", bufs=2) as wk,
            tc.tile_pool(name="ps", bufs=1, space="PSUM") as ps,
        ):
            wblob = cpool.tile([128, BLOB_COLS], bf16, name="wblob")
            nc.sync.dma_start(out=wblob, in_=wb_d)
            bblob = cpool.tile([128, BIAS_COLS], f32, name="bblob")
            nc.sync.dma_start(out=bblob, in_=bb_d)
            tb = {}
            off = 0
            for name, rows, cols in BLOB_LAYOUT:
                tb[name] = wblob[0:rows, off:off + cols]
                off += cols
            boff = 0
            for name, rows, cols in BIAS_LAYOUT:
                tb[name] = bblob[0:rows, boff:boff + cols]
                boff += cols

            sc_ps = None
            for g in range(ng):
                sub = lambda t: slice(t * TN, (t + 1) * TN)

                nsc4 = io.tile([108, 2 * GN], bf16, tag="nsc", name=f"nsc_{g}")
                nc.sync.dma_start(out=nsc4, in_=nsc_d[g])
                nscA = nsc4[:, 0:GN]

                # Y holds the hand-term halves plus the host-side extra rows
                Y = wk.tile([128, 2 * GN], bf16, tag="Y", name=f"Y_{g}")
                nc.sync.dma_start(out=Y[108:128, 0:GN], in_=exA_d[g])
                nc.sync.dma_start(out=Y[108:113, GN:2 * GN], in_=exB_d[g])

                # --- den matmuls into one 4-bank PSUM tile (A half | B half) ---
                den_ps = ps.tile([108, 2 * GN], f32, tag="pAB", name=f"den_{g}")
                for t in range(4):
                    nc.tensor.matmul(den_ps[:, sub(t)], tb["t_denA"],
                                     nscA[0:54, sub(t)], start=True, stop=True)
                for t in range(4):
                    nc.tensor.matmul(den_ps[:, GN + t * TN:GN + (t + 1) * TN],
                                     tb["t_denB"], nscA[0:54, sub(t)],
                                     start=True, stop=True)

                # --- T = Nsc * recip(den): recip on DVE, mult on Pool ---
                rd = wk.tile([108, 2 * GN], f32, tag="rd", name=f"rd_{g}")
                nc.vector.reciprocal_approx_fast(out=rd, in_=den_ps)
                T = wk.tile([108, 2 * GN], bf16, tag="T", name=f"T_{g}")
                nc.gpsimd.tensor_tensor(T, nsc4, rd, Alu.mult)

                # --- w2 matmuls (PSUM tile reused from den) ---
                w2_ps = ps.tile([108, 2 * GN], f32, tag="pAB", name=f"w2_{g}")
                for t in range(4):
                    nc.tensor.matmul(w2_ps[:, sub(t)], tb["t_w2A"],
                                     T[:, sub(t)], start=True, stop=True)
                for t in range(4):
                    nc.tensor.matmul(w2_ps[:, GN + t * TN:GN + (t + 1) * TN],
                                     tb["t_w2B"], T[:, GN + t * TN:GN + (t + 1) * TN],
                                     start=True, stop=True)

                # --- Y = w2 * Nsc (one DVE op over both halves) ---
                nc.vector.scalar_tensor_tensor(
                    out=Y[0:108, :], in0=nsc4, scalar=0.0, in1=w2_ps,
                    op0=Alu.bypass, op1=Alu.mult)

                # --- ctx layer 1 ---
                ctx1_ps = ps.tile([128, GN], f32, tag="pC", name=f"ctx1_{g}")
                for t in range(4):
                    nc.tensor.matmul(ctx1_ps[:, sub(t)], tb["t_bigA"],
                                     Y[:, sub(t)], start=True, stop=False)
                    nc.tensor.matmul(ctx1_ps[:, sub(t)], tb["t_bigB"],
                                     Y[0:113, GN + t * TN:GN + (t + 1) * TN],
                                     start=False, stop=True)
                ctx1s = wk.tile([128, GN], bf16, tag="C", name=f"C_{g}")
                nc.scalar.activation(ctx1s, ctx1_ps, Act.Relu,
                                     bias=tb["b_ctx1"], scale=1.0)

                # --- scorer hidden (u4 reuses ctx1 banks) ---
                u4_ps = ps.tile([128, GN], f32, tag="pC", name=f"u4_{g}")
                for t in range(4):
                    nc.tensor.matmul(u4_ps[:, sub(t)], tb["t_uc4"],
                                     ctx1s[:, sub(t)], start=True, stop=True)
                u4s = wk.tile([128, GN], bf16, tag="U", name=f"U_{g}")
                nc.scalar.activation(u4s, u4_ps, Act.Copy)
                H = wk.tile([128, 5 * GN], bf16, tag="H", name=f"H_{g}")
                for hg in range(5):
                    dst = H[:, hg * GN:(hg + 1) * GN]
                    bias = tb["b_H"][:, hg:hg + 1]
                    if hg < 3:
                        nc.scalar.activation(dst, u4_ps, Act.Relu,
                                             bias=bias, scale=1.0)
                    elif hg == 3:
                        nc.vector.tensor_scalar(dst, u4s, bias, 0.0,
                                                Alu.add, Alu.max)
                    else:
                        nc.gpsimd.tensor_scalar(dst, u4s, bias, 0.0,
                                                Alu.add, Alu.max)

                # --- scores: 2 groups share one PSUM tile at bases 0/32 ---
                if g % 2 == 0:
                    sc_ps = ps.tile([52, GN], f32, tag="pS", name=f"sc_{g}")
                base = 32 * (g % 2)
                dst_ps = sc_ps[base:base + 20, :]
                for t in range(4):
                    for hg in range(5):
                        nc.tensor.matmul(dst_ps[:, sub(t)], tb[f"t_sc{hg}"],
                                         H[:, hg * GN + t * TN:hg * GN + (t + 1) * TN],
                                         start=(hg == 0), stop=(hg == 4))
                if g % 2 == 1:
                    sc_sb = wk.tile([52, GN], f32, tag="S", name=f"S_{g}")
                    nc.scalar.activation(sc_sb, sc_ps, Act.Copy)
                    nc.sync.dma_start(out=out_d[g // 2], in_=sc_sb)

    nc.finalize()
    return nc


def _get_module(bc=BC):
    key = ("mod", bc)
    if key not in _CACHE:
        _CACHE[key] = _build_module(bc)
    return _CACHE[key]


# ---------------------------------------------------------------- host prep
def _prep_data(inp):
    """Full-batch host prep: counts, tiny encoders, layout. Per-core maps."""
    hc = np.asarray(inp["hand_cards"])
    B = hc.shape[0]
    gs = np.asarray(inp["game_state"], np.float32)
    dp = np.asarray(inp["discard_pile_cards"], np.float32)
    en = np.asarray(inp["enemy_card"]).reshape(B).astype(np.int64)
    hsz = np.asarray(inp["hand_size"]).astype(np.float32)

    idx = (hc.astype(np.int64) + 54 * np.arange(B, dtype=np.int64)[:, None]).ravel()
    counts = np.bincount(idx, minlength=B * 54).reshape(B, 54)
    rlen = (1.0 / np.maximum(hsz, 1.0)).astype(np.float32)
    nsc = (counts.astype(np.float32) * rlen[:, None]).T          # [54, B]
    nsc2 = np.concatenate([nsc, nsc], axis=0).astype(BF16)       # [108, B]

    # host-side tiny encoders (game state + discard MLPs, enemy embed)
    f32 = lambda k: np.asarray(inp[k], np.float32)
    g = np.maximum(gs @ f32("gs_w1").T + f32("gs_b1"), 0.0) @ f32("gs_w2").T + f32("gs_b2")
    d = np.maximum(dp @ f32("dp_w1").T + f32("dp_b1"), 0.0) @ f32("dp_w2").T + f32("dp_b2")
    en_emb = f32("enemy_emb")
    extra = np.empty((25, B), np.float32)
    extra[0:12] = en_emb[en].T
    extra[12:18] = g.T
    extra[18:24] = d.T
    extra[24] = rlen
    extra = extra.astype(BF16)

    tables = _fold_tables(inp)
    wb, bb = _pack_blobs(tables)

    maps = []
    for c in range(N_CORES):
        cols = slice(c * BC, (c + 1) * BC)
        nsc_c = np.ascontiguousarray(nsc2[:, cols]).reshape(108, NG, GN).transpose(1, 0, 2)
        nsc4 = np.ascontiguousarray(np.concatenate([nsc_c, nsc_c], axis=2))
        ex_c = np.ascontiguousarray(extra[:, cols]).reshape(25, NG, GN)
        m = {"wblob": wb, "bblob": bb, "nsc": nsc4,
             "exA": np.ascontiguousarray(ex_c[0:20].transpose(1, 0, 2)),
             "exB": np.ascontiguousarray(ex_c[20:25].transpose(1, 0, 2))}
        maps.append(m)
    return maps


def _finish_output(raw_cores, inp):
    """raw [NSG, 116, GN] per core -> [B, 20] with final bias and mask."""
    nva = int(np.asarray(inp["num_valid_actions"]).reshape(-1)[0])
    sc_b2 = float(np.asarray(inp["sc_b2"]).reshape(-1)[0])
    out = np.empty((B_FULL, A), np.float32)
    for c, raw in enumerate(raw_cores):        # raw [NG//2, 52, GN]
        for sg in range(NG // 2):
            for q in range(2):
                g = 2 * sg + q
                cols = slice(c * BC + g * GN, c * BC + (g + 1) * GN)
                out[cols] = raw[sg, 32 * q:32 * q + 20].T
    out += sc_b2
    if nva < A:
        out[:, nva:] = -1e8
    return np.ascontiguousarray(out)


# ---------------------------------------------------------------- entry points
def _enable_ldw_opt():
    # Dedup/pipeline PE weight loads between consecutive same-weight matmuls.
    import concourse.bass_utils as bu
    if getattr(bu, "_ldw_opt_patched", False):
        return
    orig = bu.run_command

    def patched(argv, **kw):
        argv = [a.replace("--enable-ldw-opt=false", "--enable-ldw-opt=true")
                if isinstance(a, str) else a for a in argv]
        return orig(argv, **kw)

    bu.run_command = patched
    bu._ldw_opt_patched = True


def _run(inputs, trace=False):
    from concourse.bass_utils import run_bass_kernel_spmd

    in_maps = _prep_data(inputs)
    nc = _get_module()
    res = run_bass_kernel_spmd(nc, in_maps, list(range(N_CORES)), trace=trace)
    out = _finish_output([r["out"] for r in res.results], inputs)
    return out, res


def kernel(**inputs) -> np.ndarray:
    out, _ = _run(inputs, trace=False)
    return out
